# revision 1
# baseline (speedup 1.0000x reference)
"""ConvolvedAttention (sliding-window causal attention, W=33) on 8 TRN2 NeuronCores.

Sharding: sequence L=8192 split 8 ways (1024 tokens/core), data-parallel over
cores. Host passes each core its query shard plus key/value shards with a
32-token halo on the left; projections are replicated. Each core runs a fused
Bass/Tile kernel: qkv projections -> banded scores (k-major, query-aligned
128-key supers) -> masked softmax -> AV -> out-projection. Host folds in the
output biases and reassembles.
"""

import numpy as np

# ---- problem constants (hardcoded per contract) ----
L, N, E = 8192, 2, 256
H, HD = 8, 32
WHALF = 32            # window//2 ; attended span = 33 (past only)
NCORES = 8
T = L // NCORES       # 1024 tokens per core
TL = 128 + T          # local tokens per batch entry: 96 pad + 32 halo + 1024
NEG = -1e9
NSUP = 9              # supers 0..8 ; super 0 = pad+halo block

# wpack column layout (fp32 cols per partition)
_WQ = 0               # 4 tiles [128,128]  (ki*2+ko)
_WK = 512
_WV = 1024            # 2 tiles [128,256]  (ki)
_WO = 1536            # 2 tiles [128,256]  (g = E_in chunk)
_BQ = 2048            # 2 cols  (ko)
_BK = 2050            # 2 cols
_BD = 2052            # [8,256] block-diag indicator (2 groups of 128 cols)
_I128 = 2308          # [128,128] identity
_MMAIN = 2436         # [128,320] band mask, 2 heads tiled (additive 0/-1e9)
_M0 = 2756            # [128,64] super-0 mask (pad+halo), 2 heads tiled
_ONES32 = 2820        # [128,32] all-ones (S-sum lhsT)
_WPCOLS = 2852

_STATE = {}


def _sup_w(s):
    return 32 if s == 0 else (128 if s == NSUP - 1 else 160)


def _build_program():
    import os
    level = int(os.environ.get("KBUILD_LEVEL", "9"))
    import concourse.bacc as bacc
    import concourse.tile as tile
    import concourse.mybir as mybir
    from contextlib import ExitStack

    f32 = mybir.dt.float32
    AF = mybir.ActivationFunctionType

    nc = bacc.Bacc("TRN2", target_bir_lowering=False, debug=False)
    xq_d = nc.declare_dram_parameter("xq", [2, 128, 2 * T], f32, isOutput=False)
    xk_d = nc.declare_dram_parameter("xk", [2, 128, 2 * TL], f32, isOutput=False)
    xv_d = nc.declare_dram_parameter("xv", [2, 128, 2 * TL], f32, isOutput=False)
    wp_d = nc.declare_dram_parameter("wpack", [128, _WPCOLS], f32, isOutput=False)
    out_d = nc.declare_dram_parameter("out", [2, 8, 128, 256], f32, isOutput=True)

    ones_col = nc.const_aps.tensor(1.0, (128, 1))

    with ExitStack() as stk:
        tc = stk.enter_context(tile.TileContext(nc))
        sb = stk.enter_context(tc.tile_pool(name="sb", bufs=1))
        sb_probs = stk.enter_context(tc.tile_pool(name="probs", bufs=2))
        sb_tr = stk.enter_context(tc.tile_pool(name="tr", bufs=3))

        # ---- load inputs ----
        wp = sb.tile([128, _WPCOLS], f32, tag="wp")
        nc.sync.dma_start(wp[:], wp_d[:])
        xq = []
        xk = []
        xv = []
        for ki in range(2):
            t_q = sb.tile([128, 2 * T], f32, tag=f"xq{ki}", name=f"xq{ki}")
            nc.sync.dma_start(t_q[:], xq_d[ki])
            xq.append(t_q)
            t_k = sb.tile([128, 2 * TL], f32, tag=f"xk{ki}", name=f"xk{ki}")
            nc.sync.dma_start(t_k[:], xk_d[ki])
            xk.append(t_k)
            t_v = sb.tile([128, 2 * TL], f32, tag=f"xv{ki}", name=f"xv{ki}")
            nc.sync.dma_start(t_v[:], xv_d[ki])
            xv.append(t_v)

        q_sb = [sb.tile([128, 2 * T], f32, tag=f"q{ko}", name=f"q{ko}") for ko in range(2)]
        k_sb = [sb.tile([128, 2 * TL], f32, tag=f"k{ko}", name=f"k{ko}") for ko in range(2)]
        v_sb = [sb.tile([128, 256], f32, tag=f"v{b}", name=f"v{b}") for b in range(2 * NSUP)]

        # ---- phase 1: projections ----
        with tc.tile_pool(name="pp", bufs=3, space="PSUM") as pp:
            # q / k projections: out [E_out chunk, tokens]
            for ko in range(2):
                bq_ap = wp[:, _BQ + ko : _BQ + ko + 1]
                bk_ap = wp[:, _BK + ko : _BK + ko + 1]
                for g0 in range(0, 2 * T, 512):
                    ps = pp.tile([128, 512], f32, tag="pq", name="pq")
                    for ki in range(2):
                        nc.tensor.matmul(
                            ps[:],
                            wp[:, _WQ + (ki * 2 + ko) * 128 : _WQ + (ki * 2 + ko + 1) * 128],
                            xq[ki][:, g0 : g0 + 512],
                            start=(ki == 0),
                            stop=(ki == 1),
                        )
                    nc.scalar.activation(
                        q_sb[ko][:, g0 : g0 + 512], ps[:], AF.Identity, bias=bq_ap
                    )
                for g0 in range(0, 2 * TL, 512):
                    w = min(512, 2 * TL - g0)
                    ps = pp.tile([128, 512], f32, tag="pq", name="pq")
                    for ki in range(2):
                        nc.tensor.matmul(
                            ps[:, :w],
                            wp[:, _WK + (ki * 2 + ko) * 128 : _WK + (ki * 2 + ko + 1) * 128],
                            xk[ki][:, g0 : g0 + w],
                            start=(ki == 0),
                            stop=(ki == 1),
                        )
                    nc.scalar.activation(
                        k_sb[ko][:, g0 : g0 + w], ps[:, :w], AF.Identity, bias=bk_ap
                    )
            # v projection: out [tokens, E_out]
            for b in range(2 * NSUP):
                ps = pp.tile([128, 256], f32, tag="pv", name="pv")
                for ki in range(2):
                    nc.tensor.matmul(
                        ps[:],
                        xv[ki][:, b * 128 : (b + 1) * 128],
                        wp[:, _WV + ki * 256 : _WV + (ki + 1) * 256],
                        start=(ki == 0),
                        stop=(ki == 1),
                    )
                nc.vector.tensor_copy(v_sb[b][:], ps[:])

        # ---- phase 2: attention ----
        i128 = wp[:, _I128 : _I128 + 128]
        mmain = wp[:, _MMAIN : _MMAIN + 320].rearrange("p (t w) -> p t w", t=2)
        m0 = wp[:, _M0 : _M0 + 64]

        with (
            tc.tile_pool(name="psc", bufs=4, space="PSUM") as psc,
            tc.tile_pool(name="pav", bufs=2, space="PSUM") as pav,
            tc.tile_pool(name="pms", bufs=2, space="PSUM") as pms,
        ):
            for n in range(2):
                probs = {}
                for s in range(NSUP):
                    w = _sup_w(s)
                    qs = 0 if s == 0 else 128 * (s - 1)
                    pr = sb_probs.tile([128, 8 * 160], f32, tag="probs", name="probs")
                    probs[s] = pr
                    if level < 2:
                        continue
                    pr_r = pr[:, : 8 * w].rearrange("p (a b) -> p a b", a=8)
                    for j in range(4):
                        # bank j: heads j and j+4, both at row group 32j
                        sc = psc.tile([128, 2 * 160], f32, tag="sc", name="sc")
                        if s == 0:
                            nc.tensor.matmul(
                                sc[:, : 2 * w], i128, m0,
                                start=True, stop=False, skip_group_check=True,
                            )
                        else:
                            nc.tensor.matmul(
                                sc[:, : 2 * w], i128, mmain[:, :, :w],
                                start=True, stop=False, skip_group_check=True,
                            )
                        for hb in range(2):
                            h = j + 4 * hb
                            ch, hr = hb, 32 * j
                            nc.tensor.matmul(
                                sc[:, hb * w : (hb + 1) * w],
                                k_sb[ch][hr : hr + 32, n * TL + 128 * s : n * TL + 128 * s + 128],
                                q_sb[ch][hr : hr + 32, n * T + qs : n * T + qs + w],
                                start=False, stop=(hb == 1),
                                tile_position=(hr, 0), skip_group_check=True,
                            )
                        nc.scalar.activation(
                            pr_r[:, j::4, :], sc[:, : 2 * w], AF.Exp,
                        )
                    if s == 0 or level < 3:
                        continue
                    # finalize query block a = s-1 (queries 128a .. 128a+128)
                    a = s - 1
                    wp_prev = _sup_w(s - 1)
                    wc = min(w, 128)
                    pcur, pprev = probs[s], probs[s - 1]
                    sps = pms.tile([128, 256], f32, tag="ms", name="ms")
                    ones32 = wp[:, _ONES32 : _ONES32 + 32]
                    for h in range(8):
                        hp, hc = 32 * (h % 4), 128 * (h // 4)
                        nc.tensor.matmul(
                            sps[hp : hp + 32, hc : hc + wc], ones32,
                            pcur[:, h * w : h * w + wc],
                            start=True, stop=False, skip_group_check=True,
                            tile_position=(0, hp),
                        )
                        nc.tensor.matmul(
                            sps[hp : hp + 32, hc : hc + 32], ones32,
                            pprev[:, h * wp_prev + wp_prev - 32 : h * wp_prev + wp_prev],
                            start=False, stop=True, skip_group_check=True,
                            tile_position=(0, hp),
                        )
                    s_sb = sb_tr.tile([128, 256], f32, tag="ssb", name="ssb")
                    nc.vector.tensor_copy(s_sb[:], sps[:])
                    s_r = sb_tr.tile([128, 256], f32, tag="sr", name="sr")
                    nc.vector.reciprocal_approx_fast(out=s_r[:], in_=s_sb[:])
                    if level < 4:
                        o_sb = sb_tr.tile([128, 256], f32, tag="osb", name="osb")
                        nc.vector.tensor_copy(o_sb[:], s_r[:])
                        nc.sync.dma_start(out_d[n, a], o_sb[:])
                        continue
                    avn = []
                    for g in range(2):
                        av = pav.tile([128, 128], f32, tag="av", name="av")
                        for hb in range(4):
                            h = 4 * g + hb
                            hr = 32 * hb
                            nc.tensor.matmul(
                                av[hr : hr + 32, :wc],
                                v_sb[NSUP * n + s][:, 32 * h : 32 * h + 32],
                                pcur[:, h * w : h * w + wc],
                                start=True, stop=False,
                                tile_position=(0, hr), skip_group_check=True,
                            )
                            nc.tensor.matmul(
                                av[hr : hr + 32, :32],
                                v_sb[NSUP * n + s - 1][:, 32 * h : 32 * h + 32],
                                pprev[:, h * wp_prev + wp_prev - 32 : h * wp_prev + wp_prev],
                                start=False, stop=True,
                                tile_position=(0, hr), skip_group_check=True,
                            )
                        t_avn = sb_tr.tile([128, 128], f32, tag="avn", name="avn")
                        if level >= 5:
                            nc.vector.tensor_mul(t_avn[:], av[:], s_r[:, 128 * g : 128 * (g + 1)])
                        else:
                            nc.vector.tensor_copy(t_avn[:], av[:])
                        avn.append(t_avn)
                    op = pms.tile([128, 256], f32, tag="ms", name="ms")
                    for g in range(2):
                        nc.tensor.matmul(
                            op[:], avn[g][:],
                            wp[:, _WO + g * 256 : _WO + (g + 1) * 256],
                            start=(g == 0), stop=(g == 1),
                        )
                    o_sb = sb_tr.tile([128, 256], f32, tag="osb", name="osb")
                    nc.scalar.copy(o_sb[:], op[:])
                    nc.sync.dma_start(out_d[n, a], o_sb[:])
                    del probs[s - 1]
                if level < 3:
                    for a in range(8):
                        o_sb = sb_tr.tile([128, 256], f32, tag="osb", name="osb")
                        if level >= 2:
                            nc.vector.tensor_copy(o_sb[:], probs[a][:, :256])
                        else:
                            nc.vector.tensor_copy(o_sb[:], v_sb[a][:])
                        nc.sync.dma_start(out_d[n, a], o_sb[:])
    nc.compile()
    return nc


def _host_prep(query, key, value, in_proj_w, in_proj_b, out_proj_w, out_proj_b):
    """Build per-core input maps + the host-side output bias vector."""
    s = 1.0 / np.sqrt(HD)
    wq = (in_proj_w[:E] * s).astype(np.float32)
    wk = in_proj_w[E : 2 * E].astype(np.float32)
    wv = in_proj_w[2 * E :].astype(np.float32)
    bq = (in_proj_b[:E] * s).astype(np.float32)
    bk = in_proj_b[E : 2 * E].astype(np.float32)
    bv = in_proj_b[2 * E :].astype(np.float32)
    wo = out_proj_w.astype(np.float32)

    wpack_base = np.zeros((128, _WPCOLS), np.float32)
    wqT, wkT = wq.T.copy(), wk.T.copy()   # [E_in, E_out]
    for ki in range(2):
        for ko in range(2):
            wpack_base[:, _WQ + (ki * 2 + ko) * 128 : _WQ + (ki * 2 + ko + 1) * 128] = \
                wqT[ki * 128 : (ki + 1) * 128, ko * 128 : (ko + 1) * 128]
            wpack_base[:, _WK + (ki * 2 + ko) * 128 : _WK + (ki * 2 + ko + 1) * 128] = \
                wkT[ki * 128 : (ki + 1) * 128, ko * 128 : (ko + 1) * 128]
        wpack_base[:, _WV + ki * 256 : _WV + (ki + 1) * 256] = \
            wv.T[ki * 128 : (ki + 1) * 128, :]
        wpack_base[:, _WO + ki * 256 : _WO + (ki + 1) * 256] = \
            wo.T[ki * 128 : (ki + 1) * 128, :]
    for ko in range(2):
        wpack_base[:, _BQ + ko] = bq[ko * 128 : (ko + 1) * 128]
        wpack_base[:, _BK + ko] = bk[ko * 128 : (ko + 1) * 128]
    # block-diag indicator [8, 256]: row k, col 128g+p -> 1 iff k == 4g + p//32
    for g in range(2):
        for hh in range(4):
            wpack_base[4 * g + hh, _BD + 128 * g + 32 * hh : _BD + 128 * g + 32 * (hh + 1)] = 1.0
    wpack_base[:128, _I128 : _I128 + 128] = np.eye(128, dtype=np.float32)
    wpack_base[:, _ONES32 : _ONES32 + 32] = 1.0
    # band mask [128, 2x160]: valid iff 0 <= c - rho <= WHALF
    rho = np.arange(128)[:, None]
    c = np.arange(160)[None, :]
    band = np.where((c - rho >= 0) & (c - rho <= WHALF), 0.0, NEG).astype(np.float32)
    wpack_base[:, _MMAIN : _MMAIN + 160] = band
    wpack_base[:, _MMAIN + 160 : _MMAIN + 320] = band

    # super-0 mask [128, 2x32]: rows 0..96 pad -> NEG ; rows 96..128 halo
    m0 = np.full((128, 64), NEG, np.float32)
    i = np.arange(32)[:, None]
    qt = np.arange(32)[None, :]
    tri = np.where(qt <= i, 0.0, NEG).astype(np.float32)
    m0[96:128, 0:32] = tri
    m0[96:128, 32:64] = tri

    qf = np.ascontiguousarray(query.transpose(2, 1, 0).astype(np.float32))  # [E, N, L]
    kf = np.ascontiguousarray(key.transpose(2, 1, 0).astype(np.float32))
    vf = np.ascontiguousarray(value.transpose(2, 1, 0).astype(np.float32))

    in_maps = []
    for cidx in range(NCORES):
        l0 = cidx * T
        xq = qf[:, :, l0 : l0 + T].reshape(2, 128, N * T)
        xk = np.zeros((2, 128, N, TL), np.float32)
        xv = np.zeros((2, 128, N, TL), np.float32)
        kfc = kf.reshape(2, 128, N, L)
        vfc = vf.reshape(2, 128, N, L)
        xk[:, :, :, 128:] = kfc[:, :, :, l0 : l0 + T]
        xv[:, :, :, 128:] = vfc[:, :, :, l0 : l0 + T]
        if cidx > 0:
            xk[:, :, :, 96:128] = kfc[:, :, :, l0 - 32 : l0]
            xv[:, :, :, 96:128] = vfc[:, :, :, l0 - 32 : l0]
        wpack = wpack_base.copy()
        if cidx == 0:
            wpack[:, _M0 : _M0 + 64] = NEG
        else:
            wpack[:, _M0 : _M0 + 64] = m0
        in_maps.append(
            {
                "xq": np.ascontiguousarray(xq),
                "xk": np.ascontiguousarray(xk.reshape(2, 128, N * TL)),
                "xv": np.ascontiguousarray(xv.reshape(2, 128, N * TL)),
                "wpack": wpack,
            }
        )
    add_vec = (out_proj_b + bv @ wo.T).astype(np.float32)
    return in_maps, add_vec


def _get_state():
    if "nc" not in _STATE:
        _STATE["nc"] = _build_program()
    return _STATE["nc"]


def kernel(query, key, value, in_proj_w, in_proj_b, out_proj_w, out_proj_b,
           collect_intermediates=0, _trace=False):
    from concourse.bass_utils import run_bass_kernel_spmd

    nc = _get_state()
    in_maps, add_vec = _host_prep(
        np.asarray(query), np.asarray(key), np.asarray(value),
        np.asarray(in_proj_w), np.asarray(in_proj_b),
        np.asarray(out_proj_w), np.asarray(out_proj_b),
    )
    res = run_bass_kernel_spmd(nc, in_maps, list(range(NCORES)), trace=_trace)
    out = np.empty((L, N, E), np.float32)
    for cidx in range(NCORES):
        dev = res.results[cidx]["out"]  # [2, 8, 128, 256]
        shard = dev.transpose(1, 2, 0, 3).reshape(T, N, E)
        out[cidx * T : (cidx + 1) * T] = shard
    out += add_vec
    if _trace:
        _STATE["last_exec_ns"] = res.exec_time_ns
        _STATE["last_res"] = res
    return out



# revision 21
# speedup vs baseline: 1.4896x; 1.4896x over previous
"""ConvolvedAttention (sliding-window causal attention, W=33) on 8 TRN2 NeuronCores.

Sequence L=8192 split 8 ways (1024 tokens/core), batch N=2 handled per core.
All matmuls in bf16 (PE at 1 cycle/col instead of fp32's 4). Attention is
tiled as stride-96 query tiles against 128-key windows so each query's full
33-key causal band lives in a single tile: no cross-tile softmax combining.
Band masking is applied multiplicatively post-exp on DVE; q-bias rides the
Act-engine PSUM evacuation; k-bias is dropped entirely (a per-query-constant
score shift is softmax-invariant); v-bias and out-bias fold into a host-side
additive vector.
"""

import numpy as np

# ---- problem constants (hardcoded per contract) ----
L, N, E = 8192, 2, 256
H, HD = 8, 32
WHALF = 32            # window//2 ; attended span = 33 (past only)
NCORES = 8
T = L // NCORES       # 1024 tokens per core per batch entry
PAD = 32              # left halo / right zero-pad on k/v
TLP = PAD + T + PAD   # 1088 padded local k/v tokens
QT = 96               # query-tile width
NT = 11               # tiles per batch entry (10x96 + 64)

# wpack_bf column layout (bf16 cols per partition)
_WQ = 0               # 4 tiles [128,128]  (ki*2+ko)
_WK = 512             # 4 tiles [128,128]
_WV = 1024            # 2 tiles [128,256]  (ki)
_WO = 1536            # 2 tiles [128,256]  (g = E_in chunk)
_ONES = 2048          # [128,32] all-ones (sums lhsT)
_BAND = 2080          # [128,16*96] band mask replicated per slot (t>0)
_BAND0 = 3616         # [128,16*96] first-tile mask (per-core content)
_WBF_COLS = 5152

_STATE = {}


def _build_program():
    import os
    import concourse.bacc as bacc
    import concourse.tile as tile
    import concourse.mybir as mybir
    from contextlib import ExitStack

    level = int(os.environ.get("KBUILD_LEVEL", "7"))

    f32 = mybir.dt.float32
    bf16 = mybir.dt.bfloat16
    AF = mybir.ActivationFunctionType

    nc = bacc.Bacc("TRN2", target_bir_lowering=False, debug=False)
    xq_d = nc.declare_dram_parameter("xq", [2, 128, N * T], bf16, isOutput=False)
    xk_d = nc.declare_dram_parameter("xk", [2, 128, N * TLP], bf16, isOutput=False)
    xv_d = nc.declare_dram_parameter("xv", [2, 128, N * TLP], bf16, isOutput=False)
    wb_d = nc.declare_dram_parameter("wb", [128, _WBF_COLS], bf16, isOutput=False)
    wf_d = nc.declare_dram_parameter("wf", [128, 2], f32, isOutput=False)
    out_d = nc.declare_dram_parameter("out", [NT, QT, N, E], f32, isOutput=True)

    with ExitStack() as stk:
        tc = stk.enter_context(tile.TileContext(nc))
        sb = stk.enter_context(tc.tile_pool(name="sb", bufs=1))
        sb_pr = stk.enter_context(tc.tile_pool(name="pr", bufs=2))
        sb_v = stk.enter_context(tc.tile_pool(name="vt", bufs=2))
        sb_r = stk.enter_context(tc.tile_pool(name="rr", bufs=2))

        # ---- input loads ----
        wb = sb.tile([128, _WBF_COLS], bf16, tag="wb")
        nc.sync.dma_start(wb[:], wb_d[:])
        wf = sb.tile([128, 2], f32, tag="wf")
        nc.sync.dma_start(wf[:], wf_d[:])
        xq, xk, xv = [], [], []
        for ki in range(2):
            t_q = sb.tile([128, N * T], bf16, tag=f"xq{ki}", name=f"xq{ki}")
            nc.sync.dma_start(t_q[:], xq_d[ki])
            xq.append(t_q)
            t_k = sb.tile([128, N * TLP], bf16, tag=f"xk{ki}", name=f"xk{ki}")
            nc.sync.dma_start(t_k[:], xk_d[ki])
            xk.append(t_k)
            t_v = sb.tile([128, N * TLP], bf16, tag=f"xv{ki}", name=f"xv{ki}")
            nc.sync.dma_start(t_v[:], xv_d[ki])
            xv.append(t_v)

        q_sb = [sb.tile([128, N * T], bf16, tag=f"q{ko}", name=f"q{ko}") for ko in range(2)]
        k_sb = [sb.tile([128, N * TLP], bf16, tag=f"k{ko}", name=f"k{ko}") for ko in range(2)]

        # ---- phase A: q/k projections (1024-col chunks, 2 psum banks each) ----
        with tc.tile_pool(name="pp", bufs=3, space="PSUM") as pp:
            for ko in range(2):
                bq_ap = wf[:, ko : ko + 1]
                for g0 in range(0, N * T, 1024):
                    ps = pp.tile([128, 1024], f32, tag="pq", name="pq")
                    for half in range(2):
                        h0 = g0 + half * 512
                        for ki in range(2):
                            nc.tensor.matmul(
                                ps[:, half * 512 : half * 512 + 512],
                                wb[:, _WQ + (ki * 2 + ko) * 128 : _WQ + (ki * 2 + ko + 1) * 128],
                                xq[ki][:, h0 : h0 + 512],
                                start=(ki == 0), stop=(ki == 1),
                                skip_group_check=True,
                            )
                    nc.scalar.activation(
                        q_sb[ko][:, g0 : g0 + 1024], ps[:], AF.Identity, bias=bq_ap
                    )
                for g0 in range(0, N * TLP, 1024):
                    w = min(1024, N * TLP - g0)
                    ps = pp.tile([128, 1024], f32, tag="pq", name="pq")
                    for half in range(0, w, 512):
                        hw = min(512, w - half)
                        for ki in range(2):
                            nc.tensor.matmul(
                                ps[:, half : half + hw],
                                wb[:, _WK + (ki * 2 + ko) * 128 : _WK + (ki * 2 + ko + 1) * 128],
                                xk[ki][:, g0 + half : g0 + half + hw],
                                start=(ki == 0), stop=(ki == 1),
                                skip_group_check=True,
                            )
                    nc.vector.tensor_copy(k_sb[ko][:, g0 : g0 + w], ps[:, :w])

        # ---- phase B: attention tiles ----
        ones32 = wb[:, _ONES : _ONES + 32]
        with (
            tc.tile_pool(name="psc", bufs=1, space="PSUM") as psc,
            tc.tile_pool(name="pav", bufs=1, space="PSUM") as pav,
            tc.tile_pool(name="pvo", bufs=1, space="PSUM") as pvo,
            tc.tile_pool(name="pout", bufs=1, space="PSUM") as pout,
        ):
            for t in range(NT):
                q0 = QT * t
                qw = min(QT, T - q0)
                # V projection for both batch entries: out [128 keys, 2*256]
                vo = pvo.tile([128, 512], f32, tag="vo", name="vo")
                for n in range(2):
                    for ki in range(2):
                        nc.tensor.matmul(
                            vo[:, 256 * n : 256 * n + 256],
                            xv[ki][:, n * TLP + q0 : n * TLP + q0 + 128],
                            wb[:, _WV + ki * 256 : _WV + (ki + 1) * 256],
                            start=(ki == 0), stop=(ki == 1),
                            skip_group_check=True,
                        )
                v_t = sb_v.tile([128, 512], bf16, tag="vt", name="vt")
                if t % 2 == 0:
                    nc.scalar.copy(v_t[:], vo[:])
                else:
                    nc.vector.tensor_copy(v_t[:], vo[:])
                if level < 2:
                    o_sb = sb_r.tile([QT, 512], f32, tag="osb", name="osb")
                    nc.vector.tensor_copy(o_sb[:qw], vo[:qw])
                    nc.sync.dma_start(out_d[t, :qw], o_sb[:qw])
                    continue

                # scores: 16 slots of 128 cols: slot (n,h) at 128*(8n+h)
                sc = psc.tile([128, 2048], f32, tag="sc", name="sc")
                for n in range(2):
                    for h in range(H):
                        hr = 32 * (h % 4)
                        ch = h // 4
                        s0 = 128 * (8 * n + h)
                        if level == 22:
                            s0 = 128 * (8 * n + h) % 512
                            nc.tensor.matmul(
                                sc[:, s0 : s0 + qw],
                                k_sb[ch][hr : hr + 32, n * TLP + q0 : n * TLP + q0 + 128],
                                q_sb[ch][hr : hr + 32, n * T + q0 : n * T + q0 + qw],
                                start=True, stop=True,
                                tile_position=(hr, 0), skip_group_check=True,
                            )
                        elif level == 24:
                            nc.tensor.matmul(
                                sc[:, s0 : s0 + qw],
                                k_sb[ch][0:32, n * TLP + q0 : n * TLP + q0 + 128],
                                q_sb[ch][0:32, n * T + q0 : n * T + q0 + qw],
                                start=True, stop=True,
                                tile_position=(0, 0), skip_group_check=True,
                            )
                        elif level == 25:
                            nc.tensor.matmul(
                                sc[:96, s0 : s0 + qw],
                                k_sb[ch][hr : hr + 32, n * TLP + q0 : n * TLP + q0 + 96],
                                q_sb[ch][hr : hr + 32, n * T + q0 : n * T + q0 + qw],
                                start=True, stop=True,
                                tile_position=(hr, 0), skip_group_check=True,
                            )
                            nc.tensor.matmul(
                                sc[96:128, s0 : s0 + qw],
                                k_sb[ch][hr : hr + 32, n * TLP + q0 + 96 : n * TLP + q0 + 128],
                                q_sb[ch][hr : hr + 32, n * T + q0 : n * T + q0 + qw],
                                start=True, stop=True,
                                tile_position=(hr, 96), skip_group_check=True,
                            )
                        elif level == 23:
                            nc.tensor.matmul(
                                sc[:, s0 : s0 + qw],
                                k_sb[ch][:, n * TLP + q0 : n * TLP + q0 + 128],
                                q_sb[ch][:, n * T + q0 : n * T + q0 + qw],
                                start=True, stop=True,
                                skip_group_check=True,
                            )
                        else:
                            # 96+32 col split dodges the FWL path, which faults
                            # for row-offset 32-row bf16 weight tiles
                            nc.tensor.matmul(
                                sc[:96, s0 : s0 + qw],
                                k_sb[ch][hr : hr + 32, n * TLP + q0 : n * TLP + q0 + 96],
                                q_sb[ch][hr : hr + 32, n * T + q0 : n * T + q0 + qw],
                                start=True, stop=True,
                                tile_position=(hr, 0), skip_group_check=True,
                            )
                            nc.tensor.matmul(
                                sc[96:128, s0 : s0 + qw],
                                k_sb[ch][hr : hr + 32, n * TLP + q0 + 96 : n * TLP + q0 + 128],
                                q_sb[ch][hr : hr + 32, n * T + q0 : n * T + q0 + qw],
                                start=True, stop=True,
                                tile_position=(hr, 96), skip_group_check=True,
                            )

                if level in (20, 22, 23, 24, 25):
                    o_sb = sb_r.tile([QT, 512], f32, tag="osb", name="osb")
                    nc.vector.tensor_copy(o_sb[:qw], sc[:qw, :512])
                    nc.sync.dma_start(out_d[t, :qw], o_sb[:qw])
                    continue

                # exp: one Act instruction over all 16 slots
                probs = sb_pr.tile([128, 16 * QT], bf16, tag="probs", name="probs")
                sc3 = sc[:].rearrange("p (s c) -> p s c", s=16)[:, :, :qw]
                pr3 = probs[:].rearrange("p (s c) -> p s c", s=16)[:, :, :qw]
                if level == 21:
                    for s in range(16):
                        nc.scalar.activation(
                            probs[:, QT * s : QT * s + qw],
                            sc[:, 128 * s : 128 * s + qw],
                            AF.Exp,
                        )
                else:
                    nc.scalar.activation(pr3, sc3, AF.Exp)

                if level < 3:
                    o_sb = sb_r.tile([QT, 512], f32, tag="osb", name="osb")
                    nc.vector.tensor_copy(o_sb[:qw], probs[:qw, :512])
                    nc.sync.dma_start(out_d[t, :qw], o_sb[:qw])
                    continue

                # band mask, multiplicative (invalid probs -> 0)
                b0 = _BAND0 if t == 0 else _BAND
                mdt = f32 if level == 33 else bf16
                pm = sb_pr.tile([128, 16 * QT], mdt, tag="pm", name="pm")
                pm3 = pm[:].rearrange("p (s c) -> p s c", s=16)[:, :, :qw]
                if qw == QT:
                    if level == 32:
                        for qtr in range(4):
                            nc.vector.tensor_mul(
                                pm[:, 384 * qtr : 384 * qtr + 384],
                                probs[:, 384 * qtr : 384 * qtr + 384],
                                wb[:, b0 + 384 * qtr : b0 + 384 * qtr + 384],
                            )
                    elif level == 34:
                        nc.gpsimd.tensor_mul(
                            pm[:], probs[:], wb[:, b0 : b0 + 16 * QT]
                        )
                    else:
                        nc.vector.tensor_mul(
                            pm[:], probs[:], wb[:, b0 : b0 + 16 * QT]
                        )
                else:
                    b3 = wb[:, b0 : b0 + 16 * QT].rearrange(
                        "p (s c) -> p s c", s=16
                    )[:, :, :qw]
                    if level == 34:
                        nc.gpsimd.tensor_mul(pm3, pr3, b3)
                    else:
                        nc.vector.tensor_mul(pm3, pr3, b3)
                probs = pm
                if level in (31, 32, 33, 34):
                    o_sb = sb_r.tile([QT, 512], f32, tag="osb", name="osb")
                    nc.vector.tensor_copy(o_sb[:qw], probs[:qw, :512])
                    nc.sync.dma_start(out_d[t, :qw], o_sb[:qw])
                    continue

                # AV + sums: av slot (n,g) at 128*(2n+g); sums at 512+same
                av = pav.tile([128, 1024], f32, tag="av", name="av")
                for n in range(2):
                    for h in range(H):
                        hb = h % 4
                        g = h // 4
                        pr_s = probs[:, QT * (8 * n + h) : QT * (8 * n + h) + qw]
                        a0 = 128 * (2 * n + g)
                        nc.tensor.matmul(
                            av[32 * hb : 32 * hb + 32, a0 : a0 + qw],
                            v_t[:, 256 * n + 32 * h : 256 * n + 32 * h + 32],
                            pr_s,
                            start=True, stop=True,
                            tile_position=(0, 32 * hb), skip_group_check=True,
                        )
                        nc.tensor.matmul(
                            av[32 * hb : 32 * hb + 32, 512 + a0 : 512 + a0 + qw],
                            ones32,
                            pr_s,
                            start=True, stop=True,
                            tile_position=(0, 32 * hb), skip_group_check=True,
                        )

                if level < 4:
                    o_sb = sb_r.tile([QT, 512], f32, tag="osb", name="osb")
                    nc.vector.tensor_copy(o_sb[:qw], av[:qw, :512])
                    nc.sync.dma_start(out_d[t, :qw], o_sb[:qw])
                    continue

                # reciprocal of sums, then normalize av -> avn (bf16)
                s_r = sb_r.tile([128, 4 * QT], f32, tag="sr", name="sr")
                sums3 = av[:, 512:].rearrange("p (s c) -> p s c", s=4)[:, :, :qw]
                sr3 = s_r[:].rearrange("p (s c) -> p s c", s=4)[:, :, :qw]
                nc.vector.reciprocal_approx_fast(out=sr3, in_=sums3)
                if level < 5:
                    o_sb = sb_r.tile([QT, 512], f32, tag="osb", name="osb")
                    nc.vector.tensor_copy(o_sb[:qw, :384], s_r[:qw])
                    nc.sync.dma_start(out_d[t, :qw], o_sb[:qw])
                    continue
                avn = sb_r.tile([128, 4 * QT], bf16, tag="avn", name="avn")
                av3 = av[:, :512].rearrange("p (s c) -> p s c", s=4)[:, :, :qw]
                avn3 = avn[:].rearrange("p (s c) -> p s c", s=4)[:, :, :qw]
                nc.vector.tensor_mul(avn3, av3, sr3)
                if level < 6:
                    o_sb = sb_r.tile([QT, 512], f32, tag="osb", name="osb")
                    nc.vector.tensor_copy(o_sb[:qw, :384], avn[:qw])
                    nc.sync.dma_start(out_d[t, :qw], o_sb[:qw])
                    continue

                # out projection: out [qw, 2*256]
                po = pout.tile([QT, 512], f32, tag="po", name="po")
                for n in range(2):
                    for g in range(2):
                        nc.tensor.matmul(
                            po[:qw, 256 * n : 256 * n + 256],
                            avn[:, QT * (2 * n + g) : QT * (2 * n + g) + qw],
                            wb[:, _WO + g * 256 : _WO + (g + 1) * 256],
                            start=(g == 0), stop=(g == 1),
                            skip_group_check=True,
                        )
                o_sb = sb_r.tile([QT, 512], f32, tag="osb", name="osb")
                if t % 2 == 0:
                    nc.vector.tensor_copy(o_sb[:qw], po[:qw])
                else:
                    nc.scalar.copy(o_sb[:qw], po[:qw])
                nc.sync.dma_start(out_d[t, :qw], o_sb[:qw])
    nc.compile()
    return nc


def _host_prep(query, key, value, in_proj_w, in_proj_b, out_proj_w, out_proj_b):
    import ml_dtypes

    bf = ml_dtypes.bfloat16
    s = 1.0 / np.sqrt(HD)
    wq = (in_proj_w[:E] * s).astype(np.float32)
    bq = (in_proj_b[:E] * s).astype(np.float32)
    wk = in_proj_w[E : 2 * E].astype(np.float32)
    wv = in_proj_w[2 * E :].astype(np.float32)
    bv = in_proj_b[2 * E :].astype(np.float32)
    wo = out_proj_w.astype(np.float32)

    wb_base = np.zeros((128, _WBF_COLS), np.float32)
    wqT, wkT = wq.T.copy(), wk.T.copy()   # [E_in, E_out]
    for ki in range(2):
        for ko in range(2):
            wb_base[:, _WQ + (ki * 2 + ko) * 128 : _WQ + (ki * 2 + ko + 1) * 128] = \
                wqT[ki * 128 : (ki + 1) * 128, ko * 128 : (ko + 1) * 128]
            wb_base[:, _WK + (ki * 2 + ko) * 128 : _WK + (ki * 2 + ko + 1) * 128] = \
                wkT[ki * 128 : (ki + 1) * 128, ko * 128 : (ko + 1) * 128]
        wb_base[:, _WV + ki * 256 : _WV + (ki + 1) * 256] = \
            wv.T[ki * 128 : (ki + 1) * 128, :]
        wb_base[:, _WO + ki * 256 : _WO + (ki + 1) * 256] = \
            wo.T[ki * 128 : (ki + 1) * 128, :]
    wb_base[:, _ONES : _ONES + 32] = 1.0

    # band mask [128, 96]: key row rho (padded coords), query col c:
    # valid iff c <= rho <= c + WHALF
    rho = np.arange(128)[:, None]
    c = np.arange(QT)[None, :]
    band = ((rho >= c) & (rho <= c + WHALF)).astype(np.float32)
    band_rep = np.tile(band, (1, 16))
    band0_c0 = np.tile(band * (rho >= PAD), (1, 16))  # core 0: no halo
    wb_base[:, _BAND : _BAND + 16 * QT] = band_rep

    wf = np.zeros((128, 2), np.float32)
    for ko in range(2):
        wf[:, ko] = bq[ko * 128 : (ko + 1) * 128]

    qf = np.ascontiguousarray(query.transpose(2, 1, 0)).astype(bf)   # [E, N, L]
    kf = np.ascontiguousarray(key.transpose(2, 1, 0)).astype(bf)
    vf = np.ascontiguousarray(value.transpose(2, 1, 0)).astype(bf)

    in_maps = []
    for cidx in range(NCORES):
        l0 = cidx * T
        xq = qf[:, :, l0 : l0 + T].reshape(2, 128, N * T)
        xk = np.zeros((2, 128, N, TLP), bf)
        xv = np.zeros((2, 128, N, TLP), bf)
        kfc = kf.reshape(2, 128, N, L)
        vfc = vf.reshape(2, 128, N, L)
        xk[:, :, :, PAD : PAD + T] = kfc[:, :, :, l0 : l0 + T]
        xv[:, :, :, PAD : PAD + T] = vfc[:, :, :, l0 : l0 + T]
        if cidx > 0:
            xk[:, :, :, :PAD] = kfc[:, :, :, l0 - PAD : l0]
            xv[:, :, :, :PAD] = vfc[:, :, :, l0 - PAD : l0]
        wb = wb_base.copy()
        wb[:, _BAND0 : _BAND0 + 16 * QT] = band0_c0 if cidx == 0 else band_rep
        in_maps.append(
            {
                "xq": np.ascontiguousarray(xq),
                "xk": np.ascontiguousarray(xk.reshape(2, 128, N * TLP)),
                "xv": np.ascontiguousarray(xv.reshape(2, 128, N * TLP)),
                "wb": wb.astype(bf),
                "wf": wf,
            }
        )
    add_vec = (out_proj_b + bv @ wo.T).astype(np.float32)
    return in_maps, add_vec


def _get_state():
    if "nc" not in _STATE:
        _STATE["nc"] = _build_program()
    return _STATE["nc"]


def kernel(query, key, value, in_proj_w, in_proj_b, out_proj_w, out_proj_b,
           collect_intermediates=0, _trace=False):
    from concourse.bass_utils import run_bass_kernel_spmd

    nc = _get_state()
    in_maps, add_vec = _host_prep(
        np.asarray(query), np.asarray(key), np.asarray(value),
        np.asarray(in_proj_w), np.asarray(in_proj_b),
        np.asarray(out_proj_w), np.asarray(out_proj_b),
    )
    res = run_bass_kernel_spmd(nc, in_maps, list(range(NCORES)), trace=_trace)
    out = np.empty((L, N, E), np.float32)
    for cidx in range(NCORES):
        dev = res.results[cidx]["out"]  # [NT, QT, N, E]
        l0 = cidx * T
        for t in range(NT):
            q0 = QT * t
            qw = min(QT, T - q0)
            out[l0 + q0 : l0 + q0 + qw] = dev[t, :qw]
    out += add_vec
    if _trace:
        _STATE["last_exec_ns"] = res.exec_time_ns
        _STATE["last_res"] = res
    return out
